# revision 17
# baseline (speedup 1.0000x reference)
"""Trainium2 Bass kernel for a custom Jacobi-basis layer.

Math:
    t = tanh(x)                                  x: [B, I] f32
    J[b,i,k] = P_k^(1,1)(t[b,i])                 Jacobi polys, k = 0..8
    out[b,o] = sum_{i,k} J[b,i,k] * coeff[o,i,k] * weights[o,i]

Strategy (8 NeuronCores, data-parallel over batch):
  * Fold weights into coeff on host: Cw[o,i,k] = coeff[o,i,k]*weights[o,i].
  * alpha=beta=1 makes the three-term recurrence two-term coefficient-free
    after rescaling: G_1 = t, G_k = t*G_{k-1} - B'_k*G_{k-2} with G_k = c_k*J_k.
    The 1/c_k scale is folded into the (host-prepared) matmul operand.
  * J_0 == 1, so the k=0 term is a per-output bias, applied with a K=1 matmul.
  * Everything on-chip runs in fp16 (measured end-to-end rel-err ~2.7e-3 vs
    the 2e-2 budget).  Engine split, chosen from measured op rates:
      - ScalarE: tanh (quarter planes) + the h_k = -B'_k*G_{k-2} scalings
        (activations run 1x on fp16) + two PSUM evictions.
      - VectorE: sq = t*t, g2 = sq - B2 (tensor_scalar runs 4x fp16),
        u_k = t*G_{k-1}, G_k = u_k + h_k (tensor_tensor runs 2x fp16;
        scalar_tensor_tensor has NO fp16 uop and is avoided), all at
        half-plane granularity so planes stream to the PE early.
  * HAM discipline: the PE clock-gate (K=4/8 -> 8/8 at 2.4GHz) only flips
    after a ~3.4us window of *uninterrupted* PE busy.  The stream is
    N=128 junk MMs (source: an uninitialized raw SBUF tensor, so they have
    no producer and start the moment the PE sequencer is live) -> K=1 bias
    MMs -> dep-pinned jitter junk -> the 128-MM real stream with no gap,
    so the flip happens once, early.
  * DMA: descriptor gen costs ~620ns *per dma_start* on the issuing
    sequencer and completion semaphores cost ~0.5-1us, so the input
    transfers are issued dep-free in priority order (consts, x/r1
    interleaved, bulk planes) on the sync HWDGE ring -- a dep link costs
    ~3us end-to-end, and the scalar (ACT) ring is unusable for inputs
    because the 1.3us ACT_TABLE_LOAD fetch serializes ahead of the data.
    Output DMAs are split across both rings (scalar ring stores its own
    evictions with no cross-engine hop, sync stores the vector-evicted
    ones).  Output is fp16; host upcasts to f32.
"""

import numpy as np

import concourse.mybir as mybir
import concourse.tile as tile
from concourse import bacc
from concourse.bass_utils import run_bass_kernel_spmd

ORDER = 8
B, I, O = 4096, 512, 512
NCORES = 8
BC = B // NCORES          # batch rows per core = 512
P = 128                   # partitions
NIC = I // P              # i-chunks = 4
BT = BC // P              # b-tiles per core = 4
FREE = NIC * BC           # free dim of basis planes = 2048
NJUNK_PRE = 18            # HAM warmup junk matmuls before the bias MMs
NJUNK_POST = 6            # junk filler between bias and the real stream


def _consts():
    """Recurrence constants (alpha=beta=1, so the k2 term is 0)."""
    a = b = 1.0
    A, Bk = {}, {}
    for i in range(2, ORDER + 1):
        A[i] = (2 * i + a + b) * (2 * i + a + b - 1) / (2 * i * (i + a + b))
        Bk[i] = (i + a - 1) * (i + b - 1) * (2 * i + a + b) / (
            i * (i + a + b) * (2 * i + a + b - 2)
        )
    c = {0: 1.0, 1: 0.5}
    for i in range(2, ORDER + 1):
        c[i] = c[i - 1] / A[i]
    Bp = {i: Bk[i] * c[i] / c[i - 2] for i in range(2, ORDER + 1)}
    return c, Bp


def _build_module():
    nc = bacc.Bacc("TRN2", num_devices=NCORES)
    f16 = mybir.dt.float16
    f32 = mybir.dt.float32

    H = FREE // 2
    Q = FREE // 4
    xt0_d = nc.dram_tensor("xt0", [P, H], f16, kind="ExternalInput")
    xt1_d = nc.dram_tensor("xt1", [P, H], f16, kind="ExternalInput")
    # r layout: [p, k*FREE + ic*O + o] = Cw[o, ic*128+p, k+1] / c_{k+1}
    r_d = nc.dram_tensor("r", [P, ORDER * FREE], f16, kind="ExternalInput")
    # consts single row: [ones(128) | bias(512)]
    consts_d = nc.dram_tensor("consts", [1, P + O], f16, kind="ExternalInput")
    # out layout: [p, bt*O + o] = output[core*BC + bt*128 + p, o]  (fp16)
    out_d = nc.dram_tensor("out", [P, BT * O], f16, kind="ExternalOutput")

    _, Bp = _consts()
    mult = mybir.AluOpType.mult
    add = mybir.AluOpType.add

    # Junk source for the HAM warmup matmuls: raw, uninitialized SBUF.
    # The values are irrelevant (results go to a never-read PSUM bank), and
    # having no producer lets the junk MMs start the moment the PE sequencer
    # reaches them -- the earliest possible start of the HAM busy window.
    junk_s = nc.alloc_sbuf_tensor("junk", [P, P], mybir.dt.float16)
    junk_ap = junk_s.ap()

    with tile.TileContext(nc) as tc:
        with (
            tc.tile_pool(name="io", bufs=1) as io,
            tc.tile_pool(name="g", bufs=1) as gp,
            tc.tile_pool(name="u", bufs=2) as up,
            tc.tile_pool(name="psum", bufs=1, space="PSUM") as pp,
        ):
            halves = (slice(0, H), slice(H, FREE))

            # ---- DMA: priority order, no dep chains ------------------------
            # all inputs on the sync ring, gen order = need order; the scalar
            # (ACT) ring is left free for the ACT_TABLE_LOAD fetch + output
            # stores (input data there serializes behind the 1.3us table).
            const_t = io.tile([1, P + O], f16, tag="consts")
            x_t = io.tile([P, FREE], f16, tag="x")
            r_t = [io.tile([P, FREE], f16, tag=f"r{k}", name=f"r{k}")
                   for k in range(ORDER)]
            nc.sync.dma_start(x_t[:, 0:H], xt0_d[:])
            nc.sync.dma_start(const_t[:], consts_d[:])
            ones_t = const_t[0:1, 0:P]
            bias_t = const_t[0:1, P : P + O]
            nc.sync.dma_start(r_t[0][:, 0:H], r_d[:, 0:H])
            nc.sync.dma_start(x_t[:, H:FREE], xt1_d[:])
            nc.sync.dma_start(r_t[0][:, H:FREE], r_d[:, H:FREE])
            for k in range(1, ORDER):
                nc.sync.dma_start(r_t[k][:], r_d[:, k * FREE : (k + 1) * FREE])

            # ---- PE warmup: junk MMs, N=128, gapless -----------------------
            ps_warm = pp.tile([P, P], f32, tag="warm", name="ps_warm")
            for _ in range(NJUNK_PRE):
                nc.tensor.matmul(
                    ps_warm[:], junk_ap, junk_ap, start=True, stop=True
                )

            # ---- fp16 basis planes -----------------------------------------
            g = [None] * (ORDER + 1)
            t = gp.tile([P, FREE], f16, tag="t")
            for h in (0, 1):
                sl = halves[h]
                nc.scalar.activation(
                    t[:, sl], x_t[:, sl],
                    mybir.ActivationFunctionType.Tanh,
                )
            g[1] = t

            # sq/g2 on DVE (tensor_tensor / tensor_scalar run 2x/4x fp16),
            # halves; the rest of the chain full-plane exactly as proven:
            # h_k = -B'_k*G_{k-2} on ScalarE, u_k/G_k tensor_tensor on DVE.
            sq = gp.tile([P, FREE], f16, tag="sq")
            g2 = gp.tile([P, FREE], f16, tag="g2")
            g[2] = g2
            for h in (0, 1):
                sl = halves[h]
                nc.vector.tensor_tensor(sq[:, sl], t[:, sl], t[:, sl], mult)
                nc.vector.tensor_scalar_add(g2[:, sl], sq[:, sl], -Bp[2])

            hk = [None] * (ORDER + 1)
            for k in range(3, ORDER + 1):
                hk[k] = gp.tile([P, FREE], f16, tag="h", name=f"h{k}", bufs=6)
                g[k] = gp.tile([P, FREE], f16, tag="g", name=f"g{k}", bufs=6)
            for k in range(3, ORDER + 1):
                u = up.tile([P, FREE], f16, tag="u", name=f"u{k}")
                nc.scalar.mul(hk[k][:], g[k - 2][:], -Bp[k])
                nc.vector.tensor_tensor(u[:], t[:], g[k - 1][:], mult)
                nc.vector.tensor_tensor(g[k][:], u[:], hk[k][:], add)

            # ---- matmul stream (gapless behind the junk) -------------------
            psums = [
                pp.tile([P, O], f32, tag=f"ps{bt}", name=f"ps{bt}")
                for bt in range(BT)
            ]
            last_bias = None
            for bt in range(BT):
                last_bias = nc.tensor.matmul(
                    psums[bt][:], ones_t, bias_t, start=True, stop=False
                )
            # jitter absorber: keep the PE busy across the bias->real handoff
            # (a gap here resets the HAM busy window, costing ~3.4us of
            # half-rate matmuls).  The dep edge pins them after the bias MMs
            # -- without it the tile scheduler hoists them earlier.
            from concourse.tile_rust import add_dep_helper
            prev = last_bias
            for _ in range(NJUNK_POST):
                j = nc.tensor.matmul(
                    ps_warm[:], junk_ap, junk_ap, start=True, stop=True
                )
                add_dep_helper(j.ins, prev.ins, reason="post-bias jitter")
                prev = j
            out_t = io.tile([P, BT * O], f16, tag="out")
            for k in range(1, ORDER + 1):
                if k < ORDER:
                    for ic in range(NIC):
                        for bt in range(BT):
                            col = ic * BC + bt * P
                            nc.tensor.matmul(
                                psums[bt][:],
                                g[k][:, col : col + P],
                                r_t[k - 1][:, ic * O : (ic + 1) * O],
                                start=False,
                                stop=False,
                            )
                else:
                    # last plane: finish b-tiles one at a time so the psum
                    # evictions/stores overlap the remaining matmuls
                    for bt in range(BT):
                        for ic in range(NIC):
                            col = ic * BC + bt * P
                            nc.tensor.matmul(
                                psums[bt][:],
                                g[k][:, col : col + P],
                                r_t[k - 1][:, ic * O : (ic + 1) * O],
                                start=False,
                                stop=ic == NIC - 1,
                            )
                        dst = out_t[:, bt * O : (bt + 1) * O]
                        if bt < BT - 1:
                            if bt % 2 == 0:
                                nc.scalar.copy(dst, psums[bt][:])
                                nc.scalar.dma_start(
                                    out_d[:, bt * O : (bt + 1) * O], dst
                                )
                            else:
                                nc.vector.tensor_copy(dst, psums[bt][:])
                                nc.sync.dma_start(
                                    out_d[:, bt * O : (bt + 1) * O], dst
                                )
                        else:
                            # last tile: split halves across both engines and
                            # both HWDGE rings
                            hw = O // 2
                            nc.scalar.copy(
                                out_t[:, bt * O : bt * O + hw],
                                psums[bt][:, 0:hw],
                            )
                            nc.scalar.dma_start(
                                out_d[:, bt * O : bt * O + hw],
                                out_t[:, bt * O : bt * O + hw],
                            )
                            nc.vector.tensor_copy(
                                out_t[:, bt * O + hw : (bt + 1) * O],
                                psums[bt][:, hw:O],
                            )
                            nc.sync.dma_start(
                                out_d[:, bt * O + hw : (bt + 1) * O],
                                out_t[:, bt * O + hw : (bt + 1) * O],
                            )
    nc.compile()
    return nc


def _prep_operands(weights, coeff):
    """Host-side, input-independent preprocessing of the layer constants."""
    c, _ = _consts()
    Cw = coeff.astype(np.float64) * weights.astype(np.float64)[:, :, None]
    bias = Cw[:, :, 0].sum(axis=1)                      # [O]
    r = np.empty((ORDER, P, FREE), dtype=np.float32)
    for k in range(1, ORDER + 1):
        tmp = (Cw[:, :, k] / c[k]).T.astype(np.float32)  # [I, O]
        r[k - 1] = tmp.reshape(NIC, P, O).transpose(1, 0, 2).reshape(P, FREE)
    r = np.ascontiguousarray(
        r.transpose(1, 0, 2).reshape(P, ORDER * FREE)
    ).astype(np.float16)
    consts = np.empty((1, P + O), dtype=np.float32)
    consts[0, :P] = 1.0
    consts[0, P:] = bias
    return r, consts.astype(np.float16)


def _prep_x(x):
    """Per-core [2, 128, FREE/2] fp16 views of x^T:
    xt[p, ic*BC + b] = x[core*BC+b, ic*128+p]."""
    shards = []
    for core in range(NCORES):
        xc = np.ascontiguousarray(x[core * BC : (core + 1) * BC, :].T)  # [I, BC]
        flat = xc.reshape(NIC, P, BC).transpose(1, 0, 2).reshape(P, FREE)
        halves = np.ascontiguousarray(
            flat.reshape(P, 2, FREE // 2).transpose(1, 0, 2)
        ).astype(np.float16)
        shards.append((halves[0], halves[1]))
    return shards


def _install_ntff_hook():
    """Register the NTFF profile hook that the image's boot skips (no
    antenv.axon_hooks module). Same ctypes ABI as trn_boot's
    _ntff_profile_via_ctypes. Only used for traced (profiling) runs."""
    import sys
    import types
    import ctypes
    import contextlib

    if "antenv.axon_hooks" in sys.modules:
        return
    mod = types.ModuleType("antenv.axon_hooks")
    state = {"hook": None}
    mod.set_axon_ntff_profile_hook = lambda h: state.__setitem__("hook", h)
    mod.get_axon_ntff_profile_hook = lambda: state["hook"]
    sys.modules["antenv.axon_hooks"] = mod
    import antenv

    antenv.axon_hooks = mod

    so_path = "/opt/axon/libaxon_pjrt.so"
    lib = ctypes.CDLL(so_path)
    if not hasattr(lib, "axon_start_nrt_profile"):
        return
    lib.axon_start_nrt_profile.argtypes = [
        ctypes.POINTER(ctypes.c_int64),
        ctypes.c_size_t,
    ]
    lib.axon_start_nrt_profile.restype = ctypes.c_int64
    lib.axon_stop_nrt_profile.argtypes = [ctypes.c_char_p]
    lib.axon_stop_nrt_profile.restype = ctypes.c_int64

    @contextlib.contextmanager
    def _hook(output_dir, device_ids):
        import jax

        jax.devices()
        if device_ids:
            ids = (ctypes.c_int64 * len(device_ids))(*device_ids)
            rc = lib.axon_start_nrt_profile(ids, len(device_ids))
        else:
            rc = lib.axon_start_nrt_profile(None, 0)
        if rc != 0:
            raise RuntimeError(f"axon_start_nrt_profile rc={rc}")
        try:
            yield
        finally:
            n = lib.axon_stop_nrt_profile(str(output_dir).encode())
            print(f"ntff profile: {n} file(s) written to {output_dir}")

    mod.set_axon_ntff_profile_hook(_hook)


_NC_CACHE = None


def _get_module():
    global _NC_CACHE
    if _NC_CACHE is None:
        _NC_CACHE = _build_module()
    return _NC_CACHE


def _run(x, weights, coeff, trace=False):
    nc = _get_module()
    r, consts = _prep_operands(weights, coeff)
    xs = _prep_x(np.asarray(x, dtype=np.float32))
    in_maps = [
        {"xt0": xs[core][0], "xt1": xs[core][1], "r": r, "consts": consts}
        for core in range(NCORES)
    ]
    try:
        res = run_bass_kernel_spmd(
            nc, in_maps, core_ids=list(range(NCORES)), trace=trace
        )
    except Exception:
        res = run_bass_kernel_spmd(
            nc, in_maps, core_ids=list(range(NCORES)), trace=trace
        )
    out = np.concatenate(
        [
            res.results[core]["out"]
            .astype(np.float32)
            .reshape(P, BT, O)
            .transpose(1, 0, 2)
            .reshape(BC, O)
            for core in range(NCORES)
        ],
        axis=0,
    )
    return out, res


def kernel(x, weights, coeff):
    out, _ = _run(x, weights, coeff, trace=False)
    return out


def kernel_traced(x, weights, coeff):
    _install_ntff_hook()
    out, res = _run(x, weights, coeff, trace=True)
    return out, res


# revision 18
# speedup vs baseline: 1.0274x; 1.0274x over previous
"""Trainium2 Bass kernel for a custom Jacobi-basis layer.

Math:
    t = tanh(x)                                  x: [B, I] f32
    J[b,i,k] = P_k^(1,1)(t[b,i])                 Jacobi polys, k = 0..8
    out[b,o] = sum_{i,k} J[b,i,k] * coeff[o,i,k] * weights[o,i]

Strategy (8 NeuronCores, data-parallel over batch):
  * Fold weights into coeff on host: Cw[o,i,k] = coeff[o,i,k]*weights[o,i].
  * alpha=beta=1 makes the three-term recurrence two-term coefficient-free
    after rescaling: G_1 = t, G_k = t*G_{k-1} - B'_k*G_{k-2} with G_k = c_k*J_k.
    The 1/c_k scale is folded into the (host-prepared) matmul operand.
  * J_0 == 1, so the k=0 term is a per-output bias, applied with a K=1 matmul.
  * Everything on-chip runs in fp16 (measured end-to-end rel-err ~2.7e-3 vs
    the 2e-2 budget).  Engine split, chosen from measured op rates:
      - ScalarE: tanh (quarter planes) + the h_k = -B'_k*G_{k-2} scalings
        (activations run 1x on fp16) + two PSUM evictions.
      - VectorE: sq = t*t, g2 = sq - B2 (tensor_scalar runs 4x fp16),
        u_k = t*G_{k-1}, G_k = u_k + h_k (tensor_tensor runs 2x fp16;
        scalar_tensor_tensor has NO fp16 uop and is avoided), all at
        half-plane granularity so planes stream to the PE early.
  * HAM discipline: the PE clock-gate (K=4/8 -> 8/8 at 2.4GHz) only flips
    after a ~3.4us window of *uninterrupted* PE busy.  The stream is
    N=128 junk MMs (source: an uninitialized raw SBUF tensor, so they have
    no producer and start the moment the PE sequencer is live) -> K=1 bias
    MMs -> dep-pinned jitter junk -> the 128-MM real stream with no gap,
    so the flip happens once, early.
  * DMA: descriptor gen costs ~620ns *per dma_start* on the issuing
    sequencer and completion semaphores cost ~0.5-1us, so the input
    transfers are issued dep-free in priority order (consts, x/r1
    interleaved, bulk planes) on the sync HWDGE ring -- a dep link costs
    ~3us end-to-end, and the scalar (ACT) ring is unusable for inputs
    because the 1.3us ACT_TABLE_LOAD fetch serializes ahead of the data.
    Output DMAs are split across both rings (scalar ring stores its own
    evictions with no cross-engine hop, sync stores the vector-evicted
    ones).  Output is fp16; host upcasts to f32.
"""

import numpy as np

import concourse.mybir as mybir
import concourse.tile as tile
from concourse import bacc
from concourse.bass_utils import run_bass_kernel_spmd

ORDER = 8
B, I, O = 4096, 512, 512
NCORES = 8
BC = B // NCORES          # batch rows per core = 512
P = 128                   # partitions
NIC = I // P              # i-chunks = 4
BT = BC // P              # b-tiles per core = 4
FREE = NIC * BC           # free dim of basis planes = 2048
NJUNK_PRE = 18            # HAM warmup junk matmuls before the bias MMs
NJUNK_POST = 6            # junk filler between bias and the real stream


def _consts():
    """Recurrence constants (alpha=beta=1, so the k2 term is 0)."""
    a = b = 1.0
    A, Bk = {}, {}
    for i in range(2, ORDER + 1):
        A[i] = (2 * i + a + b) * (2 * i + a + b - 1) / (2 * i * (i + a + b))
        Bk[i] = (i + a - 1) * (i + b - 1) * (2 * i + a + b) / (
            i * (i + a + b) * (2 * i + a + b - 2)
        )
    c = {0: 1.0, 1: 0.5}
    for i in range(2, ORDER + 1):
        c[i] = c[i - 1] / A[i]
    Bp = {i: Bk[i] * c[i] / c[i - 2] for i in range(2, ORDER + 1)}
    return c, Bp


def _build_module():
    nc = bacc.Bacc("TRN2", num_devices=NCORES)
    f16 = mybir.dt.float16
    f32 = mybir.dt.float32

    H = FREE // 2
    Q = FREE // 4
    xt0_d = nc.dram_tensor("xt0", [P, H], f16, kind="ExternalInput")
    xt1_d = nc.dram_tensor("xt1", [P, H], f16, kind="ExternalInput")
    # r layout: [p, k*FREE + ic*O + o] = Cw[o, ic*128+p, k+1] / c_{k+1}
    r_d = nc.dram_tensor("r", [P, ORDER * FREE], f16, kind="ExternalInput")
    # consts single row: [ones(128) | bias(512)]
    consts_d = nc.dram_tensor("consts", [1, P + O], f16, kind="ExternalInput")
    # out layout: [p, bt*O + o] = output[core*BC + bt*128 + p, o]  (fp16)
    out_d = nc.dram_tensor("out", [P, BT * O], f16, kind="ExternalOutput")

    _, Bp = _consts()
    mult = mybir.AluOpType.mult
    add = mybir.AluOpType.add

    # Junk source for the HAM warmup matmuls: raw, uninitialized SBUF.
    # The values are irrelevant (results go to a never-read PSUM bank), and
    # having no producer lets the junk MMs start the moment the PE sequencer
    # reaches them -- the earliest possible start of the HAM busy window.
    junk_s = nc.alloc_sbuf_tensor("junk", [P, P], mybir.dt.float16)
    junk_ap = junk_s.ap()

    with tile.TileContext(nc) as tc:
        with (
            tc.tile_pool(name="io", bufs=1) as io,
            tc.tile_pool(name="g", bufs=1) as gp,
            tc.tile_pool(name="u", bufs=2) as up,
            tc.tile_pool(name="psum", bufs=1, space="PSUM") as pp,
        ):
            halves = (slice(0, H), slice(H, FREE))

            # ---- DMA: priority order, no dep chains ------------------------
            # sync ring: consts, r1 halves, then bulk planes
            const_t = io.tile([1, P + O], f16, tag="consts")
            nc.sync.dma_start(const_t[:], consts_d[:])
            ones_t = const_t[0:1, 0:P]
            bias_t = const_t[0:1, P : P + O]

            # scalar ring: x halves (feeds scalar's own tanh; gen runs in
            # parallel with the sync ring's consts/r gens)
            x_t = io.tile([P, FREE], f16, tag="x")
            nc.scalar.dma_start(x_t[:, 0:H], xt0_d[:])
            nc.scalar.dma_start(x_t[:, H:FREE], xt1_d[:])

            r_t = [io.tile([P, FREE], f16, tag=f"r{k}", name=f"r{k}")
                   for k in range(ORDER)]
            nc.sync.dma_start(r_t[0][:, 0:H], r_d[:, 0:H])
            nc.sync.dma_start(r_t[0][:, H:FREE], r_d[:, H:FREE])
            for k in range(1, ORDER):
                nc.sync.dma_start(r_t[k][:], r_d[:, k * FREE : (k + 1) * FREE])

            # ---- PE warmup: junk MMs, N=128, gapless -----------------------
            ps_warm = pp.tile([P, P], f32, tag="warm", name="ps_warm")
            for _ in range(NJUNK_PRE):
                nc.tensor.matmul(
                    ps_warm[:], junk_ap, junk_ap, start=True, stop=True
                )

            # ---- fp16 basis planes -----------------------------------------
            g = [None] * (ORDER + 1)
            t = gp.tile([P, FREE], f16, tag="t")
            for h in (0, 1):
                sl = halves[h]
                nc.scalar.activation(
                    t[:, sl], x_t[:, sl],
                    mybir.ActivationFunctionType.Tanh,
                )
            g[1] = t

            # sq/g2 on DVE (tensor_tensor / tensor_scalar run 2x/4x fp16),
            # halves; the rest of the chain full-plane exactly as proven:
            # h_k = -B'_k*G_{k-2} on ScalarE, u_k/G_k tensor_tensor on DVE.
            sq = gp.tile([P, FREE], f16, tag="sq")
            g2 = gp.tile([P, FREE], f16, tag="g2")
            g[2] = g2
            for h in (0, 1):
                sl = halves[h]
                nc.vector.tensor_tensor(sq[:, sl], t[:, sl], t[:, sl], mult)
                nc.vector.tensor_scalar_add(g2[:, sl], sq[:, sl], -Bp[2])

            hk = [None] * (ORDER + 1)
            for k in range(3, ORDER + 1):
                hk[k] = gp.tile([P, FREE], f16, tag="h", name=f"h{k}", bufs=6)
                g[k] = gp.tile([P, FREE], f16, tag="g", name=f"g{k}", bufs=6)
            for k in range(3, ORDER + 1):
                u = up.tile([P, FREE], f16, tag="u", name=f"u{k}")
                nc.scalar.mul(hk[k][:], g[k - 2][:], -Bp[k])
                nc.vector.tensor_tensor(u[:], t[:], g[k - 1][:], mult)
                nc.vector.tensor_tensor(g[k][:], u[:], hk[k][:], add)

            # ---- matmul stream (gapless behind the junk) -------------------
            psums = [
                pp.tile([P, O], f32, tag=f"ps{bt}", name=f"ps{bt}")
                for bt in range(BT)
            ]
            last_bias = None
            for bt in range(BT):
                last_bias = nc.tensor.matmul(
                    psums[bt][:], ones_t, bias_t, start=True, stop=False
                )
            # jitter absorber: keep the PE busy across the bias->real handoff
            # (a gap here resets the HAM busy window, costing ~3.4us of
            # half-rate matmuls).  The dep edge pins them after the bias MMs
            # -- without it the tile scheduler hoists them earlier.
            from concourse.tile_rust import add_dep_helper
            prev = last_bias
            for _ in range(NJUNK_POST):
                j = nc.tensor.matmul(
                    ps_warm[:], junk_ap, junk_ap, start=True, stop=True
                )
                add_dep_helper(j.ins, prev.ins, reason="post-bias jitter")
                prev = j
            out_t = io.tile([P, BT * O], f16, tag="out")
            for k in range(1, ORDER + 1):
                if k < ORDER:
                    for ic in range(NIC):
                        for bt in range(BT):
                            col = ic * BC + bt * P
                            nc.tensor.matmul(
                                psums[bt][:],
                                g[k][:, col : col + P],
                                r_t[k - 1][:, ic * O : (ic + 1) * O],
                                start=False,
                                stop=False,
                            )
                else:
                    # last plane: finish b-tiles one at a time so the psum
                    # evictions/stores overlap the remaining matmuls
                    for bt in range(BT):
                        for ic in range(NIC):
                            col = ic * BC + bt * P
                            nc.tensor.matmul(
                                psums[bt][:],
                                g[k][:, col : col + P],
                                r_t[k - 1][:, ic * O : (ic + 1) * O],
                                start=False,
                                stop=ic == NIC - 1,
                            )
                        dst = out_t[:, bt * O : (bt + 1) * O]
                        if bt < BT - 1:
                            if bt % 2 == 0:
                                nc.scalar.copy(dst, psums[bt][:])
                                nc.scalar.dma_start(
                                    out_d[:, bt * O : (bt + 1) * O], dst
                                )
                            else:
                                nc.vector.tensor_copy(dst, psums[bt][:])
                                nc.sync.dma_start(
                                    out_d[:, bt * O : (bt + 1) * O], dst
                                )
                        else:
                            # last tile: split halves across both engines and
                            # both HWDGE rings
                            hw = O // 2
                            nc.scalar.copy(
                                out_t[:, bt * O : bt * O + hw],
                                psums[bt][:, 0:hw],
                            )
                            nc.scalar.dma_start(
                                out_d[:, bt * O : bt * O + hw],
                                out_t[:, bt * O : bt * O + hw],
                            )
                            nc.vector.tensor_copy(
                                out_t[:, bt * O + hw : (bt + 1) * O],
                                psums[bt][:, hw:O],
                            )
                            nc.sync.dma_start(
                                out_d[:, bt * O + hw : (bt + 1) * O],
                                out_t[:, bt * O + hw : (bt + 1) * O],
                            )
    nc.compile()
    return nc


def _prep_operands(weights, coeff):
    """Host-side, input-independent preprocessing of the layer constants."""
    c, _ = _consts()
    Cw = coeff.astype(np.float64) * weights.astype(np.float64)[:, :, None]
    bias = Cw[:, :, 0].sum(axis=1)                      # [O]
    r = np.empty((ORDER, P, FREE), dtype=np.float32)
    for k in range(1, ORDER + 1):
        tmp = (Cw[:, :, k] / c[k]).T.astype(np.float32)  # [I, O]
        r[k - 1] = tmp.reshape(NIC, P, O).transpose(1, 0, 2).reshape(P, FREE)
    r = np.ascontiguousarray(
        r.transpose(1, 0, 2).reshape(P, ORDER * FREE)
    ).astype(np.float16)
    consts = np.empty((1, P + O), dtype=np.float32)
    consts[0, :P] = 1.0
    consts[0, P:] = bias
    return r, consts.astype(np.float16)


def _prep_x(x):
    """Per-core [2, 128, FREE/2] fp16 views of x^T:
    xt[p, ic*BC + b] = x[core*BC+b, ic*128+p]."""
    shards = []
    for core in range(NCORES):
        xc = np.ascontiguousarray(x[core * BC : (core + 1) * BC, :].T)  # [I, BC]
        flat = xc.reshape(NIC, P, BC).transpose(1, 0, 2).reshape(P, FREE)
        halves = np.ascontiguousarray(
            flat.reshape(P, 2, FREE // 2).transpose(1, 0, 2)
        ).astype(np.float16)
        shards.append((halves[0], halves[1]))
    return shards


def _install_ntff_hook():
    """Register the NTFF profile hook that the image's boot skips (no
    antenv.axon_hooks module). Same ctypes ABI as trn_boot's
    _ntff_profile_via_ctypes. Only used for traced (profiling) runs."""
    import sys
    import types
    import ctypes
    import contextlib

    if "antenv.axon_hooks" in sys.modules:
        return
    mod = types.ModuleType("antenv.axon_hooks")
    state = {"hook": None}
    mod.set_axon_ntff_profile_hook = lambda h: state.__setitem__("hook", h)
    mod.get_axon_ntff_profile_hook = lambda: state["hook"]
    sys.modules["antenv.axon_hooks"] = mod
    import antenv

    antenv.axon_hooks = mod

    so_path = "/opt/axon/libaxon_pjrt.so"
    lib = ctypes.CDLL(so_path)
    if not hasattr(lib, "axon_start_nrt_profile"):
        return
    lib.axon_start_nrt_profile.argtypes = [
        ctypes.POINTER(ctypes.c_int64),
        ctypes.c_size_t,
    ]
    lib.axon_start_nrt_profile.restype = ctypes.c_int64
    lib.axon_stop_nrt_profile.argtypes = [ctypes.c_char_p]
    lib.axon_stop_nrt_profile.restype = ctypes.c_int64

    @contextlib.contextmanager
    def _hook(output_dir, device_ids):
        import jax

        jax.devices()
        if device_ids:
            ids = (ctypes.c_int64 * len(device_ids))(*device_ids)
            rc = lib.axon_start_nrt_profile(ids, len(device_ids))
        else:
            rc = lib.axon_start_nrt_profile(None, 0)
        if rc != 0:
            raise RuntimeError(f"axon_start_nrt_profile rc={rc}")
        try:
            yield
        finally:
            n = lib.axon_stop_nrt_profile(str(output_dir).encode())
            print(f"ntff profile: {n} file(s) written to {output_dir}")

    mod.set_axon_ntff_profile_hook(_hook)


_NC_CACHE = None


def _get_module():
    global _NC_CACHE
    if _NC_CACHE is None:
        _NC_CACHE = _build_module()
    return _NC_CACHE


def _run(x, weights, coeff, trace=False):
    nc = _get_module()
    r, consts = _prep_operands(weights, coeff)
    xs = _prep_x(np.asarray(x, dtype=np.float32))
    in_maps = [
        {"xt0": xs[core][0], "xt1": xs[core][1], "r": r, "consts": consts}
        for core in range(NCORES)
    ]
    try:
        res = run_bass_kernel_spmd(
            nc, in_maps, core_ids=list(range(NCORES)), trace=trace
        )
    except Exception:
        res = run_bass_kernel_spmd(
            nc, in_maps, core_ids=list(range(NCORES)), trace=trace
        )
    out = np.concatenate(
        [
            res.results[core]["out"]
            .astype(np.float32)
            .reshape(P, BT, O)
            .transpose(1, 0, 2)
            .reshape(BC, O)
            for core in range(NCORES)
        ],
        axis=0,
    )
    return out, res


def kernel(x, weights, coeff):
    out, _ = _run(x, weights, coeff, trace=False)
    return out


def kernel_traced(x, weights, coeff):
    _install_ntff_hook()
    out, res = _run(x, weights, coeff, trace=True)
    return out, res


# revision 19
# speedup vs baseline: 1.0442x; 1.0164x over previous
"""Trainium2 Bass kernel for a custom Jacobi-basis layer.

Math:
    t = tanh(x)                                  x: [B, I] f32
    J[b,i,k] = P_k^(1,1)(t[b,i])                 Jacobi polys, k = 0..8
    out[b,o] = sum_{i,k} J[b,i,k] * coeff[o,i,k] * weights[o,i]

Strategy (8 NeuronCores, data-parallel over batch):
  * Fold weights into coeff on host: Cw[o,i,k] = coeff[o,i,k]*weights[o,i].
  * alpha=beta=1 makes the three-term recurrence two-term coefficient-free
    after rescaling: G_1 = t, G_k = t*G_{k-1} - B'_k*G_{k-2} with G_k = c_k*J_k.
    The 1/c_k scale is folded into the (host-prepared) matmul operand.
  * J_0 == 1, so the k=0 term is a per-output bias, applied with a K=1 matmul.
  * Everything on-chip runs in fp16 (measured end-to-end rel-err ~2.7e-3 vs
    the 2e-2 budget).  Engine split, chosen from measured op rates:
      - ScalarE: tanh (quarter planes) + the h_k = -B'_k*G_{k-2} scalings
        (activations run 1x on fp16) + two PSUM evictions.
      - VectorE: sq = t*t, g2 = sq - B2 (tensor_scalar runs 4x fp16),
        u_k = t*G_{k-1}, G_k = u_k + h_k (tensor_tensor runs 2x fp16;
        scalar_tensor_tensor has NO fp16 uop and is avoided), all at
        half-plane granularity so planes stream to the PE early.
  * HAM discipline: the PE clock-gate (K=4/8 -> 8/8 at 2.4GHz) only flips
    after a ~3.4us window of *uninterrupted* PE busy.  The stream is
    N=128 junk MMs (source: an uninitialized raw SBUF tensor, so they have
    no producer and start the moment the PE sequencer is live) -> K=1 bias
    MMs -> dep-pinned jitter junk -> the 128-MM real stream with no gap,
    so the flip happens once, early.
  * DMA: descriptor gen costs ~620ns *per dma_start* on the issuing
    sequencer and completion semaphores cost ~0.5-1us, so the input
    transfers are issued dep-free in priority order (consts, x/r1
    interleaved, bulk planes) on the sync HWDGE ring -- a dep link costs
    ~3us end-to-end, and the scalar (ACT) ring is unusable for inputs
    because the 1.3us ACT_TABLE_LOAD fetch serializes ahead of the data.
    Output DMAs are split across both rings (scalar ring stores its own
    evictions with no cross-engine hop, sync stores the vector-evicted
    ones).  Output is fp16; host upcasts to f32.
"""

import numpy as np

import concourse.mybir as mybir
import concourse.tile as tile
from concourse import bacc
from concourse.bass_utils import run_bass_kernel_spmd

ORDER = 8
B, I, O = 4096, 512, 512
NCORES = 8
BC = B // NCORES          # batch rows per core = 512
P = 128                   # partitions
NIC = I // P              # i-chunks = 4
BT = BC // P              # b-tiles per core = 4
FREE = NIC * BC           # free dim of basis planes = 2048
NJUNK_PRE = 24            # HAM warmup junk matmuls before the bias MMs
NJUNK_POST = 3            # junk filler between bias and the real stream


def _consts():
    """Recurrence constants (alpha=beta=1, so the k2 term is 0)."""
    a = b = 1.0
    A, Bk = {}, {}
    for i in range(2, ORDER + 1):
        A[i] = (2 * i + a + b) * (2 * i + a + b - 1) / (2 * i * (i + a + b))
        Bk[i] = (i + a - 1) * (i + b - 1) * (2 * i + a + b) / (
            i * (i + a + b) * (2 * i + a + b - 2)
        )
    c = {0: 1.0, 1: 0.5}
    for i in range(2, ORDER + 1):
        c[i] = c[i - 1] / A[i]
    Bp = {i: Bk[i] * c[i] / c[i - 2] for i in range(2, ORDER + 1)}
    return c, Bp


def _build_module():
    nc = bacc.Bacc("TRN2", num_devices=NCORES)
    f16 = mybir.dt.float16
    f32 = mybir.dt.float32

    H = FREE // 2
    Q = FREE // 4
    xt0_d = nc.dram_tensor("xt0", [P, H], f16, kind="ExternalInput")
    xt1_d = nc.dram_tensor("xt1", [P, H], f16, kind="ExternalInput")
    # r layout: [p, k*FREE + ic*O + o] = Cw[o, ic*128+p, k+1] / c_{k+1}
    r_d = nc.dram_tensor("r", [P, ORDER * FREE], f16, kind="ExternalInput")
    # consts single row: [ones(128) | bias(512)]
    consts_d = nc.dram_tensor("consts", [1, P + O], f16, kind="ExternalInput")
    # out layout: [p, bt*O + o] = output[core*BC + bt*128 + p, o]  (fp16)
    out_d = nc.dram_tensor("out", [P, BT * O], f16, kind="ExternalOutput")

    _, Bp = _consts()
    mult = mybir.AluOpType.mult
    add = mybir.AluOpType.add

    # Junk source for the HAM warmup matmuls: raw, uninitialized SBUF.
    # The values are irrelevant (results go to a never-read PSUM bank), and
    # having no producer lets the junk MMs start the moment the PE sequencer
    # reaches them -- the earliest possible start of the HAM busy window.
    junk_s = nc.alloc_sbuf_tensor("junk", [P, P], mybir.dt.float16)
    junk_ap = junk_s.ap()

    with tile.TileContext(nc) as tc:
        with (
            tc.tile_pool(name="io", bufs=1) as io,
            tc.tile_pool(name="g", bufs=1) as gp,
            tc.tile_pool(name="u", bufs=2) as up,
            tc.tile_pool(name="psum", bufs=1, space="PSUM") as pp,
        ):
            halves = (slice(0, H), slice(H, FREE))

            # ---- DMA: priority order, no dep chains ------------------------
            # sync ring: consts, r1 halves, then bulk planes
            const_t = io.tile([1, P + O], f16, tag="consts")
            nc.sync.dma_start(const_t[:], consts_d[:])
            ones_t = const_t[0:1, 0:P]
            bias_t = const_t[0:1, P : P + O]

            # scalar ring: x halves (feeds scalar's own tanh; gen runs in
            # parallel with the sync ring's consts/r gens)
            x_t = io.tile([P, FREE], f16, tag="x")
            nc.scalar.dma_start(x_t[:, 0:H], xt0_d[:])
            nc.scalar.dma_start(x_t[:, H:FREE], xt1_d[:])

            r_t = [io.tile([P, FREE], f16, tag=f"r{k}", name=f"r{k}")
                   for k in range(ORDER)]
            nc.sync.dma_start(r_t[0][:, 0:H], r_d[:, 0:H])
            nc.sync.dma_start(r_t[0][:, H:FREE], r_d[:, H:FREE])
            for k in range(1, ORDER):
                nc.sync.dma_start(r_t[k][:], r_d[:, k * FREE : (k + 1) * FREE])

            # ---- PE warmup: junk MMs, N=128, gapless -----------------------
            ps_warm = pp.tile([P, P], f32, tag="warm", name="ps_warm")
            for _ in range(NJUNK_PRE):
                nc.tensor.matmul(
                    ps_warm[:], junk_ap, junk_ap, start=True, stop=True
                )

            # ---- fp16 basis planes -----------------------------------------
            g = [None] * (ORDER + 1)
            t = gp.tile([P, FREE], f16, tag="t")
            for h in (0, 1):
                sl = halves[h]
                nc.scalar.activation(
                    t[:, sl], x_t[:, sl],
                    mybir.ActivationFunctionType.Tanh,
                )
            g[1] = t

            # sq/g2 on DVE (tensor_tensor / tensor_scalar run 2x/4x fp16),
            # halves; the rest of the chain full-plane exactly as proven:
            # h_k = -B'_k*G_{k-2} on ScalarE, u_k/G_k tensor_tensor on DVE.
            sq = gp.tile([P, FREE], f16, tag="sq")
            g2 = gp.tile([P, FREE], f16, tag="g2")
            g[2] = g2
            for h in (0, 1):
                sl = halves[h]
                nc.vector.tensor_tensor(sq[:, sl], t[:, sl], t[:, sl], mult)
                nc.vector.tensor_scalar_add(g2[:, sl], sq[:, sl], -Bp[2])

            hk = [None] * (ORDER + 1)
            for k in range(3, ORDER + 1):
                hk[k] = gp.tile([P, FREE], f16, tag="h", name=f"h{k}", bufs=6)
                g[k] = gp.tile([P, FREE], f16, tag="g", name=f"g{k}", bufs=6)
            for k in range(3, ORDER + 1):
                u = up.tile([P, FREE], f16, tag="u", name=f"u{k}")
                nc.scalar.mul(hk[k][:], g[k - 2][:], -Bp[k])
                nc.vector.tensor_tensor(u[:], t[:], g[k - 1][:], mult)
                nc.vector.tensor_tensor(g[k][:], u[:], hk[k][:], add)

            # ---- matmul stream (gapless behind the junk) -------------------
            psums = [
                pp.tile([P, O], f32, tag=f"ps{bt}", name=f"ps{bt}")
                for bt in range(BT)
            ]
            last_bias = None
            for bt in range(BT):
                last_bias = nc.tensor.matmul(
                    psums[bt][:], ones_t, bias_t, start=True, stop=False
                )
            # jitter absorber: keep the PE busy across the bias->real handoff
            # (a gap here resets the HAM busy window, costing ~3.4us of
            # half-rate matmuls).  The dep edge pins them after the bias MMs
            # -- without it the tile scheduler hoists them earlier.
            from concourse.tile_rust import add_dep_helper
            prev = last_bias
            for _ in range(NJUNK_POST):
                j = nc.tensor.matmul(
                    ps_warm[:], junk_ap, junk_ap, start=True, stop=True
                )
                add_dep_helper(j.ins, prev.ins, reason="post-bias jitter")
                prev = j
            out_t = io.tile([P, BT * O], f16, tag="out")
            for k in range(1, ORDER + 1):
                if k < ORDER:
                    for ic in range(NIC):
                        for bt in range(BT):
                            col = ic * BC + bt * P
                            nc.tensor.matmul(
                                psums[bt][:],
                                g[k][:, col : col + P],
                                r_t[k - 1][:, ic * O : (ic + 1) * O],
                                start=False,
                                stop=False,
                            )
                else:
                    # last plane: finish b-tiles one at a time so the psum
                    # evictions/stores overlap the remaining matmuls
                    for bt in range(BT):
                        for ic in range(NIC):
                            col = ic * BC + bt * P
                            nc.tensor.matmul(
                                psums[bt][:],
                                g[k][:, col : col + P],
                                r_t[k - 1][:, ic * O : (ic + 1) * O],
                                start=False,
                                stop=ic == NIC - 1,
                            )
                        dst = out_t[:, bt * O : (bt + 1) * O]
                        if bt < BT - 1:
                            if bt % 2 == 0:
                                nc.scalar.copy(dst, psums[bt][:])
                                nc.scalar.dma_start(
                                    out_d[:, bt * O : (bt + 1) * O], dst
                                )
                            else:
                                nc.vector.tensor_copy(dst, psums[bt][:])
                                nc.sync.dma_start(
                                    out_d[:, bt * O : (bt + 1) * O], dst
                                )
                        else:
                            # last tile: split halves across both engines and
                            # both HWDGE rings
                            hw = O // 2
                            nc.scalar.copy(
                                out_t[:, bt * O : bt * O + hw],
                                psums[bt][:, 0:hw],
                            )
                            nc.scalar.dma_start(
                                out_d[:, bt * O : bt * O + hw],
                                out_t[:, bt * O : bt * O + hw],
                            )
                            nc.vector.tensor_copy(
                                out_t[:, bt * O + hw : (bt + 1) * O],
                                psums[bt][:, hw:O],
                            )
                            nc.sync.dma_start(
                                out_d[:, bt * O + hw : (bt + 1) * O],
                                out_t[:, bt * O + hw : (bt + 1) * O],
                            )
    nc.compile()
    return nc


def _prep_operands(weights, coeff):
    """Host-side, input-independent preprocessing of the layer constants."""
    c, _ = _consts()
    Cw = coeff.astype(np.float64) * weights.astype(np.float64)[:, :, None]
    bias = Cw[:, :, 0].sum(axis=1)                      # [O]
    r = np.empty((ORDER, P, FREE), dtype=np.float32)
    for k in range(1, ORDER + 1):
        tmp = (Cw[:, :, k] / c[k]).T.astype(np.float32)  # [I, O]
        r[k - 1] = tmp.reshape(NIC, P, O).transpose(1, 0, 2).reshape(P, FREE)
    r = np.ascontiguousarray(
        r.transpose(1, 0, 2).reshape(P, ORDER * FREE)
    ).astype(np.float16)
    consts = np.empty((1, P + O), dtype=np.float32)
    consts[0, :P] = 1.0
    consts[0, P:] = bias
    return r, consts.astype(np.float16)


def _prep_x(x):
    """Per-core [2, 128, FREE/2] fp16 views of x^T:
    xt[p, ic*BC + b] = x[core*BC+b, ic*128+p]."""
    shards = []
    for core in range(NCORES):
        xc = np.ascontiguousarray(x[core * BC : (core + 1) * BC, :].T)  # [I, BC]
        flat = xc.reshape(NIC, P, BC).transpose(1, 0, 2).reshape(P, FREE)
        halves = np.ascontiguousarray(
            flat.reshape(P, 2, FREE // 2).transpose(1, 0, 2)
        ).astype(np.float16)
        shards.append((halves[0], halves[1]))
    return shards


def _install_ntff_hook():
    """Register the NTFF profile hook that the image's boot skips (no
    antenv.axon_hooks module). Same ctypes ABI as trn_boot's
    _ntff_profile_via_ctypes. Only used for traced (profiling) runs."""
    import sys
    import types
    import ctypes
    import contextlib

    if "antenv.axon_hooks" in sys.modules:
        return
    mod = types.ModuleType("antenv.axon_hooks")
    state = {"hook": None}
    mod.set_axon_ntff_profile_hook = lambda h: state.__setitem__("hook", h)
    mod.get_axon_ntff_profile_hook = lambda: state["hook"]
    sys.modules["antenv.axon_hooks"] = mod
    import antenv

    antenv.axon_hooks = mod

    so_path = "/opt/axon/libaxon_pjrt.so"
    lib = ctypes.CDLL(so_path)
    if not hasattr(lib, "axon_start_nrt_profile"):
        return
    lib.axon_start_nrt_profile.argtypes = [
        ctypes.POINTER(ctypes.c_int64),
        ctypes.c_size_t,
    ]
    lib.axon_start_nrt_profile.restype = ctypes.c_int64
    lib.axon_stop_nrt_profile.argtypes = [ctypes.c_char_p]
    lib.axon_stop_nrt_profile.restype = ctypes.c_int64

    @contextlib.contextmanager
    def _hook(output_dir, device_ids):
        import jax

        jax.devices()
        if device_ids:
            ids = (ctypes.c_int64 * len(device_ids))(*device_ids)
            rc = lib.axon_start_nrt_profile(ids, len(device_ids))
        else:
            rc = lib.axon_start_nrt_profile(None, 0)
        if rc != 0:
            raise RuntimeError(f"axon_start_nrt_profile rc={rc}")
        try:
            yield
        finally:
            n = lib.axon_stop_nrt_profile(str(output_dir).encode())
            print(f"ntff profile: {n} file(s) written to {output_dir}")

    mod.set_axon_ntff_profile_hook(_hook)


_NC_CACHE = None


def _get_module():
    global _NC_CACHE
    if _NC_CACHE is None:
        _NC_CACHE = _build_module()
    return _NC_CACHE


def _run(x, weights, coeff, trace=False):
    nc = _get_module()
    r, consts = _prep_operands(weights, coeff)
    xs = _prep_x(np.asarray(x, dtype=np.float32))
    in_maps = [
        {"xt0": xs[core][0], "xt1": xs[core][1], "r": r, "consts": consts}
        for core in range(NCORES)
    ]
    try:
        res = run_bass_kernel_spmd(
            nc, in_maps, core_ids=list(range(NCORES)), trace=trace
        )
    except Exception:
        res = run_bass_kernel_spmd(
            nc, in_maps, core_ids=list(range(NCORES)), trace=trace
        )
    out = np.concatenate(
        [
            res.results[core]["out"]
            .astype(np.float32)
            .reshape(P, BT, O)
            .transpose(1, 0, 2)
            .reshape(BC, O)
            for core in range(NCORES)
        ],
        axis=0,
    )
    return out, res


def kernel(x, weights, coeff):
    out, _ = _run(x, weights, coeff, trace=False)
    return out


def kernel_traced(x, weights, coeff):
    _install_ntff_hook()
    out, res = _run(x, weights, coeff, trace=True)
    return out, res
